# revision 12
# baseline (speedup 1.0000x reference)
"""Trainium2 Bass kernel for nn_BitwiseLinear.

Reference semantics (B=32768, IN=OUT=1024):
    out = in_scale * weight_scale * (sign(x) @ sign(weight * gate_mask).T + bias)
    gate_mask = (sign(gate)+1)/2; in_scale = mean|x| per row; weight_scale = mean|w| per out.

Identities used:
    sign(weight * gate_mask) == sign(weight) * (gate >= 0)   (gate==0 -> mask 0.5 -> sign(w))
    out = sum|x|_row * ws_eff * (signmm + bias),  ws_eff = sum|w|_row * 2^-20

Implementation: data-parallel over batch on 8 cores, weights replicated.
Per core: sign(x) tiles are PE-transposed (f32) into PSUM, Sign-activated into
fp8; binarized weights (+-1/0 in fp8, transposed to [i, o]) feed DoubleRow fp8
matmuls (K=256 per MM). Epilogue: psum * sum|x| (ACT scale) * ws_eff (TT).
"""

import numpy as np

import concourse.bacc as bacc
import concourse.mybir as mybir
import concourse.tile as tile
from concourse import masks
from concourse.bass_utils import run_bass_kernel_spmd

B, IN, OUT = 32768, 1024, 1024
NCORES = 8
BSH = B // NCORES            # 4096 rows per core
P = 128                      # partitions
NT = BSH // P                # 32 x-tiles per core
KC = IN // P                 # 8 contraction chunks of 128
NPAIR = KC // 2              # 4 DoubleRow K-pairs (256 each)
NCH = 512                    # matmul moving free-dim (one PSUM bank of f32)
F32 = mybir.dt.float32
BF16 = mybir.dt.bfloat16
FP8 = mybir.dt.float8e4
WS_SCALE = float(2.0 ** -20)  # 1/(1024*1024): folds both mean divisors

_CACHE: dict = {}


def _build():
    nc = bacc.Bacc("TRN2", target_bir_lowering=False, debug=False,
                   num_devices=NCORES)

    x_ext = nc.declare_dram_parameter("x", [BSH, IN], F32, isOutput=False)
    w_ext = nc.declare_dram_parameter("weight", [OUT, IN], F32, isOutput=False)
    g_ext = nc.declare_dram_parameter("gate", [OUT, IN], F32, isOutput=False)
    b_ext = nc.declare_dram_parameter("bias", [1, OUT], F32, isOutput=False)
    o_ext = nc.declare_dram_parameter("out", [BSH, OUT], F32, isOutput=True)

    x_ap = x_ext.ap()
    w_ap = w_ext.ap()
    g_ap = g_ext.ap()
    b_ap = b_ext.ap()
    o_ap = o_ext.ap()

    ACT = mybir.ActivationFunctionType
    ALU = mybir.AluOpType
    AX = mybir.AxisListType
    DR = mybir.MatmulPerfMode.DoubleRow

    with tile.TileContext(nc) as tc:
        with tc.tile_pool(name="const", bufs=1) as cp:
            ident_f32 = cp.tile([P, P], F32)
            masks.make_identity(nc, ident_f32[:])
            ident_bf = cp.tile([P, P], BF16)
            masks.make_identity(nc, ident_bf[:])
            ones_f8 = cp.tile([1, P], FP8)
            nc.gpsimd.memset(ones_f8[:], 1.0)
            ones_f32 = cp.tile([1, P], F32)
            nc.gpsimd.memset(ones_f32[:], 1.0)
            zbias = cp.tile([P, 1], F32)
            nc.gpsimd.memset(zbias[:], 0.0)

            # persistent prepped weights
            # pair j holds binarized wT chunks 2j (at [:, :OUT]) and 2j+1
            wtq = [cp.tile([P, 2 * OUT], FP8, tag=f"wtq{j}", name=f"wtq{j}") for j in range(NPAIR)]
            bias_f8 = cp.tile([1, OUT], FP8)      # raw bias (fp8) added pre-scale
            ws_bcast = cp.tile([P, OUT], F32)     # ws * 2^-20 broadcast over partitions

            # ---------------- weight prep ----------------
            with tc.tile_pool(name="wprep", bufs=2) as wp, \
                 tc.tile_pool(name="wkeep", bufs=1) as wk, \
                 tc.tile_pool(name="wpsum1", bufs=1, space="PSUM") as wps1, \
                 tc.tile_pool(name="wpsum", bufs=2, space="PSUM") as wps:
                # o-tile t: [128 o_t, 1024 i]
                w_bin = [wk.tile([P, IN], BF16, tag=f"wbin{t}", name=f"wbin{t}") for t in range(KC)]
                ws_cols = wk.tile([P, KC], F32)   # per-o |w| row sums, o-tile t in col t
                bias_sb = wk.tile([1, OUT], F32)
                ws_row = wk.tile([1, OUT], F32)
                nc.sync.dma_start(bias_sb[:], b_ap[:, :])
                nc.vector.tensor_copy(bias_f8[:], bias_sb[:])

                for t in range(KC):
                    wt = wp.tile([P, IN], F32)
                    nc.sync.dma_start(wt[:], w_ap[t * P:(t + 1) * P, :])
                    gt = wp.tile([P, IN], F32)
                    nc.sync.dma_start(gt[:], g_ap[t * P:(t + 1) * P, :])
                    nc.vector.tensor_reduce(ws_cols[:, t:t + 1], wt[:], axis=AX.X,
                                            op=ALU.add, apply_absolute_value=True)
                    sgn = wp.tile([P, IN], BF16)
                    nc.scalar.activation(sgn[:], wt[:], ACT.Sign, bias=zbias[:])
                    msk = wp.tile([P, IN], BF16)
                    nc.vector.tensor_scalar(msk[:], gt[:], 0.0, None, op0=ALU.is_ge)
                    nc.vector.tensor_tensor(w_bin[t][:], sgn[:], msk[:], op=ALU.mult)

                # wtq pair j, half h = transpose(w_bin)[128 i-rows of chunk 2j+h, all o]
                for c in range(KC):
                    ps_wt = wps.tile([P, OUT], BF16, tag="ps_wt")
                    for t in range(KC):
                        nc.tensor.transpose(
                            ps_wt[:, t * P:(t + 1) * P],
                            w_bin[t][:, c * P:(c + 1) * P],
                            ident_bf[:])
                    nc.vector.tensor_copy(
                        wtq[c // 2][:, (c % 2) * OUT:((c % 2) + 1) * OUT], ps_wt[:])

                # ws_row[0, o] = sum_i |w[o, i]| * 2^-20, via 8 tiny PE transposes
                ps_row = wps1.tile([1, OUT], F32)
                for t in range(KC):
                    nc.tensor.transpose(ps_row[0:1, t * P:(t + 1) * P],
                                        ws_cols[:, t:t + 1], ident_f32[:])
                nc.scalar.activation(ws_row[:], ps_row[:], ACT.Copy, scale=WS_SCALE)

                # broadcast ws_row across partitions with a K=1 matmul
                ps_bc = wps1.tile([P, OUT], F32, tag="ps_row")
                for n in range(OUT // NCH):
                    nc.tensor.matmul(ps_bc[:, n * NCH:(n + 1) * NCH], ones_f32[:],
                                     ws_row[:, n * NCH:(n + 1) * NCH])
                nc.vector.tensor_copy(ws_bcast[:], ps_bc[:])

            # ---------------- main loop over x tiles ----------------
            with tc.tile_pool(name="xin", bufs=5) as xin_pool, \
                 tc.tile_pool(name="xbt", bufs=3) as xbt_pool, \
                 tc.tile_pool(name="osb", bufs=3) as osb_pool, \
                 tc.tile_pool(name="sc", bufs=4) as sc_pool, \
                 tc.tile_pool(name="pst", bufs=4, space="PSUM") as pst_pool, \
                 tc.tile_pool(name="pso", bufs=4, space="PSUM") as pso_pool:

                xbts = [None] * NT
                is_raws = [None] * NT

                def stage_front(it):
                    """DMA in + row abs-sum + PE transpose + fp8 sign."""
                    xt = xin_pool.tile([P, IN], F32)
                    nc.sync.dma_start(xt[:], x_ap[it * P:(it + 1) * P, :])
                    is_raw = sc_pool.tile([P, 1], F32, tag="is_raw")
                    nc.vector.tensor_reduce(is_raw[:], xt[:], axis=AX.X,
                                            op=ALU.add, apply_absolute_value=True)
                    xbT = []
                    for h in range(2):
                        ps_t = pst_pool.tile([P, NCH], F32, tag="ps_t")
                        for ci in range(KC // 2):
                            c = h * (KC // 2) + ci
                            nc.tensor.transpose(ps_t[:, ci * P:(ci + 1) * P],
                                                xt[:, c * P:(c + 1) * P],
                                                ident_f32[:])
                        xbh = xbt_pool.tile([P, NCH], FP8, tag="xbT",
                                            name=f"xbT{h}")
                        nc.scalar.activation(xbh[:], ps_t[:], ACT.Sign, bias=zbias[:])
                        xbT.append(xbh)
                    xbts[it] = xbT
                    is_raws[it] = is_raw

                out_sbs = [None] * NT

                def stage_back(it):
                    """Deferred final scale + store (runs one iter later so the
                    ACT Copy never head-of-line blocks the next Sign)."""
                    out2 = osb_pool.tile([P, OUT], F32, tag="out2")
                    nc.scalar.activation(out2[:], out_sbs[it][:], ACT.Copy,
                                         scale=is_raws[it][:])
                    nc.sync.dma_start(o_ap[it * P:(it + 1) * P, :], out2[:])

                stage_front(0)
                for it in range(NT):
                    if it + 1 < NT:
                        stage_front(it + 1)

                    xbT = xbts[it]
                    is_raw = is_raws[it]
                    out_sb = osb_pool.tile([P, OUT], F32, tag="out_sb")
                    ps_os = []
                    for n in range(OUT // NCH):
                        ps_os.append(pso_pool.tile([P, NCH], F32, tag="ps_o",
                                                   name=f"ps_o{n}"))
                    for j in range(NPAIR):
                        h, jj = divmod(j, NPAIR // 2)
                        xp = xbT[h][:, jj * 2 * P:(jj + 1) * 2 * P].rearrange(
                            "p (two m) -> p two m", two=2)
                        wq = wtq[j][:].rearrange("p (two o) -> p two o", two=2)
                        for n in range(OUT // NCH):
                            nc.tensor.matmul(
                                ps_os[n][:],
                                xp,
                                wq[:, :, n * NCH:(n + 1) * NCH],
                                start=(j == 0), stop=False, perf_mode=DR)
                    for n in range(OUT // NCH):
                        nc.tensor.matmul(ps_os[n][:], ones_f8[:],
                                         bias_f8[:, n * NCH:(n + 1) * NCH],
                                         start=False, stop=True)
                        nc.vector.tensor_tensor(out_sb[:, n * NCH:(n + 1) * NCH],
                                                ps_os[n][:],
                                                ws_bcast[:, n * NCH:(n + 1) * NCH],
                                                op=ALU.mult)
                    out_sbs[it] = out_sb

                    if it >= 1:
                        stage_back(it - 1)
                stage_back(NT - 1)

    nc.compile()
    return nc


def _get_nc():
    if "nc" not in _CACHE:
        _CACHE["nc"] = _build()
    return _CACHE["nc"]


def run(x, weight, gate, bias, trace=False):
    nc = _get_nc()
    x = np.ascontiguousarray(np.asarray(x, dtype=np.float32))
    weight = np.ascontiguousarray(np.asarray(weight, dtype=np.float32))
    gate = np.ascontiguousarray(np.asarray(gate, dtype=np.float32))
    bias = np.ascontiguousarray(np.asarray(bias, dtype=np.float32)).reshape(1, OUT)
    in_maps = [
        {"x": x[i * BSH:(i + 1) * BSH], "weight": weight, "gate": gate, "bias": bias}
        for i in range(NCORES)
    ]
    res = run_bass_kernel_spmd(nc, in_maps, core_ids=list(range(NCORES)), trace=trace)
    out = np.concatenate([res.results[i]["out"] for i in range(NCORES)], axis=0)
    return out, res


def kernel(x, weight, gate, bias):
    out, _ = run(x, weight, gate, bias, trace=False)
    return out


# revision 13
# speedup vs baseline: 1.2215x; 1.2215x over previous
"""Trainium2 Bass kernel for nn_BitwiseLinear.

Reference semantics (B=32768, IN=OUT=1024):
    out = in_scale * weight_scale * (sign(x) @ sign(weight * gate_mask).T + bias)
    gate_mask = (sign(gate)+1)/2; in_scale = mean|x| per row; weight_scale = mean|w| per out.

Identities used:
    sign(weight * gate_mask) == sign(weight) * (gate >= 0)   (gate==0 -> mask 0.5 -> sign(w))
    out = sum|x|_row * ws_eff * (signmm + bias),  ws_eff = sum|w|_row * 2^-20

Implementation: data-parallel over batch on 8 cores, weights replicated.
Per core: sign(x) tiles are PE-transposed (f32) into PSUM, Sign-activated into
fp8; binarized weights (+-1/0 in fp8, transposed to [i, o]) feed DoubleRow fp8
matmuls (K=256 per MM). Epilogue: psum * sum|x| (ACT scale) * ws_eff (TT).
"""

import numpy as np

import concourse.bacc as bacc
import concourse.mybir as mybir
import concourse.tile as tile
from concourse import masks
from concourse.bass_utils import run_bass_kernel_spmd

B, IN, OUT = 32768, 1024, 1024
NCORES = 8
BSH = B // NCORES            # 4096 rows per core
P = 128                      # partitions
NT = BSH // P                # 32 x-tiles per core
KC = IN // P                 # 8 contraction chunks of 128
NPAIR = KC // 2              # 4 DoubleRow K-pairs (256 each)
NCH = 512                    # matmul moving free-dim (one PSUM bank of f32)
F32 = mybir.dt.float32
BF16 = mybir.dt.bfloat16
FP8 = mybir.dt.float8e4
WS_SCALE = float(2.0 ** -20)  # 1/(1024*1024): folds both mean divisors

_CACHE: dict = {}


def _build(with_bias=True):
    nc = bacc.Bacc("TRN2", target_bir_lowering=False, debug=False,
                   num_devices=NCORES)

    x_ext = nc.declare_dram_parameter("x", [BSH, IN], F32, isOutput=False)
    w_ext = nc.declare_dram_parameter("weight", [OUT, IN], F32, isOutput=False)
    g_ext = nc.declare_dram_parameter("gate", [OUT, IN], F32, isOutput=False)
    b_ext = nc.declare_dram_parameter("bias", [1, OUT], F32, isOutput=False)
    o_ext = nc.declare_dram_parameter("out", [BSH, OUT], F32, isOutput=True)

    x_ap = x_ext.ap()
    w_ap = w_ext.ap()
    g_ap = g_ext.ap()
    b_ap = b_ext.ap()
    o_ap = o_ext.ap()

    ACT = mybir.ActivationFunctionType
    ALU = mybir.AluOpType
    AX = mybir.AxisListType
    DR = mybir.MatmulPerfMode.DoubleRow

    with tile.TileContext(nc) as tc:
        with tc.tile_pool(name="const", bufs=1) as cp:
            ident_f32 = cp.tile([P, P], F32)
            masks.make_identity(nc, ident_f32[:])
            ident_bf = cp.tile([P, P], BF16)
            masks.make_identity(nc, ident_bf[:])
            ones_f8 = cp.tile([1, P], FP8)
            nc.gpsimd.memset(ones_f8[:], 1.0)
            ones_f32 = cp.tile([1, P], F32)
            nc.gpsimd.memset(ones_f32[:], 1.0)
            zbias = cp.tile([P, 1], F32)
            nc.gpsimd.memset(zbias[:], 0.0)

            # persistent prepped weights
            # pair j holds binarized wT chunks 2j (at [:, :OUT]) and 2j+1
            wtq = [cp.tile([P, 2 * OUT], FP8, tag=f"wtq{j}", name=f"wtq{j}") for j in range(NPAIR)]
            bias_f8 = cp.tile([1, OUT], FP8)      # raw bias (fp8) added pre-scale
            ws_bcast = cp.tile([P, OUT], F32)     # ws * 2^-20 broadcast over partitions

            # ---------------- weight prep ----------------
            with tc.tile_pool(name="wprep", bufs=2) as wp, \
                 tc.tile_pool(name="wkeep", bufs=1) as wk, \
                 tc.tile_pool(name="wpsum1", bufs=1, space="PSUM") as wps1, \
                 tc.tile_pool(name="wpsum", bufs=2, space="PSUM") as wps:
                # o-tile t: [128 o_t, 1024 i]
                w_bin = [wk.tile([P, IN], BF16, tag=f"wbin{t}", name=f"wbin{t}") for t in range(KC)]
                ws_cols = wk.tile([P, KC], F32)   # per-o |w| row sums, o-tile t in col t
                bias_sb = wk.tile([1, OUT], F32)
                ws_row = wk.tile([1, OUT], F32)
                if with_bias:
                    nc.sync.dma_start(bias_sb[:], b_ap[:, :])
                    nc.vector.tensor_copy(bias_f8[:], bias_sb[:])

                for t in range(KC):
                    wt = wp.tile([P, IN], F32)
                    nc.sync.dma_start(wt[:], w_ap[t * P:(t + 1) * P, :])
                    gt = wp.tile([P, IN], F32)
                    nc.sync.dma_start(gt[:], g_ap[t * P:(t + 1) * P, :])
                    nc.vector.tensor_reduce(ws_cols[:, t:t + 1], wt[:], axis=AX.X,
                                            op=ALU.add, apply_absolute_value=True)
                    sgn = wp.tile([P, IN], BF16)
                    nc.scalar.activation(sgn[:], wt[:], ACT.Sign, bias=zbias[:])
                    msk = wp.tile([P, IN], BF16)
                    nc.vector.tensor_scalar(msk[:], gt[:], 0.0, None, op0=ALU.is_ge)
                    nc.vector.tensor_tensor(w_bin[t][:], sgn[:], msk[:], op=ALU.mult)

                # wtq pair j, half h = transpose(w_bin)[128 i-rows of chunk 2j+h, all o]
                for c in range(KC):
                    ps_wt = wps.tile([P, OUT], BF16, tag="ps_wt")
                    for t in range(KC):
                        nc.tensor.transpose(
                            ps_wt[:, t * P:(t + 1) * P],
                            w_bin[t][:, c * P:(c + 1) * P],
                            ident_bf[:])
                    nc.vector.tensor_copy(
                        wtq[c // 2][:, (c % 2) * OUT:((c % 2) + 1) * OUT], ps_wt[:])

                # ws_row[0, o] = sum_i |w[o, i]| * 2^-20, via 8 tiny PE transposes
                ps_row = wps1.tile([1, OUT], F32)
                for t in range(KC):
                    nc.tensor.transpose(ps_row[0:1, t * P:(t + 1) * P],
                                        ws_cols[:, t:t + 1], ident_f32[:])
                nc.scalar.activation(ws_row[:], ps_row[:], ACT.Copy, scale=WS_SCALE)

                # broadcast ws_row across partitions with a K=1 matmul
                ps_bc = wps1.tile([P, OUT], F32, tag="ps_row")
                for n in range(OUT // NCH):
                    nc.tensor.matmul(ps_bc[:, n * NCH:(n + 1) * NCH], ones_f32[:],
                                     ws_row[:, n * NCH:(n + 1) * NCH])
                nc.vector.tensor_copy(ws_bcast[:], ps_bc[:])

            # ---------------- main loop over x tiles ----------------
            with tc.tile_pool(name="xin", bufs=5) as xin_pool, \
                 tc.tile_pool(name="xbt", bufs=6) as xbt_pool, \
                 tc.tile_pool(name="osb", bufs=3) as osb_pool, \
                 tc.tile_pool(name="sc", bufs=4) as sc_pool, \
                 tc.tile_pool(name="pst", bufs=4, space="PSUM") as pst_pool, \
                 tc.tile_pool(name="pso", bufs=4, space="PSUM") as pso_pool:

                xbts = [None] * NT
                is_raws = [None] * NT

                def stage_front(it):
                    """DMA in + row abs-sum + PE transpose + fp8 sign."""
                    xt = xin_pool.tile([P, IN], F32)
                    nc.sync.dma_start(xt[:], x_ap[it * P:(it + 1) * P, :])
                    is_raw = sc_pool.tile([P, 1], F32, tag="is_raw")
                    nc.vector.tensor_reduce(is_raw[:], xt[:], axis=AX.X,
                                            op=ALU.add, apply_absolute_value=True)
                    xbT = []
                    for h in range(2):
                        ps_t = pst_pool.tile([P, NCH], F32, tag="ps_t")
                        for ci in range(KC // 2):
                            c = h * (KC // 2) + ci
                            nc.tensor.transpose(ps_t[:, ci * P:(ci + 1) * P],
                                                xt[:, c * P:(c + 1) * P],
                                                ident_f32[:])
                        xbh = xbt_pool.tile([P, NCH], FP8, tag="xbT",
                                            name=f"xbT{h}")
                        nc.scalar.activation(xbh[:], ps_t[:], ACT.Sign, bias=zbias[:])
                        xbT.append(xbh)
                    xbts[it] = xbT
                    is_raws[it] = is_raw

                out_sbs = [None] * NT

                def stage_back(it):
                    """Deferred final scale + store (runs one iter later so the
                    ACT Copy never head-of-line blocks the next Sign)."""
                    out2 = osb_pool.tile([P, OUT], F32, tag="out2")
                    nc.scalar.activation(out2[:], out_sbs[it][:], ACT.Copy,
                                         scale=is_raws[it][:])
                    nc.sync.dma_start(o_ap[it * P:(it + 1) * P, :], out2[:])

                stage_front(0)
                stage_front(1)
                for it in range(NT):
                    if it + 2 < NT:
                        stage_front(it + 2)

                    xbT = xbts[it]
                    is_raw = is_raws[it]
                    out_sb = osb_pool.tile([P, OUT], F32, tag="out_sb")
                    ps_os = []
                    for n in range(OUT // NCH):
                        ps_os.append(pso_pool.tile([P, NCH], F32, tag="ps_o",
                                                   name=f"ps_o{n}"))
                    for j in range(NPAIR):
                        h, jj = divmod(j, NPAIR // 2)
                        xp = xbT[h][:, jj * 2 * P:(jj + 1) * 2 * P].rearrange(
                            "p (two m) -> p two m", two=2)
                        wq = wtq[j][:].rearrange("p (two o) -> p two o", two=2)
                        for n in range(OUT // NCH):
                            nc.tensor.matmul(
                                ps_os[n][:],
                                xp,
                                wq[:, :, n * NCH:(n + 1) * NCH],
                                start=(j == 0),
                                stop=(not with_bias and j == NPAIR - 1),
                                perf_mode=DR)
                    for n in range(OUT // NCH):
                        if with_bias:
                            nc.tensor.matmul(ps_os[n][:], ones_f8[:],
                                             bias_f8[:, n * NCH:(n + 1) * NCH],
                                             start=False, stop=True)
                        nc.vector.tensor_tensor(out_sb[:, n * NCH:(n + 1) * NCH],
                                                ps_os[n][:],
                                                ws_bcast[:, n * NCH:(n + 1) * NCH],
                                                op=ALU.mult)
                    out_sbs[it] = out_sb

                    if it >= 1:
                        stage_back(it - 1)
                stage_back(NT - 1)

    nc.compile()
    return nc


def _get_nc(with_bias):
    key = f"nc{int(with_bias)}"
    if key not in _CACHE:
        _CACHE[key] = _build(with_bias)
    return _CACHE[key]


def run(x, weight, gate, bias, trace=False):
    nc = _get_nc(bool(np.any(np.asarray(bias))))
    x = np.ascontiguousarray(np.asarray(x, dtype=np.float32))
    weight = np.ascontiguousarray(np.asarray(weight, dtype=np.float32))
    gate = np.ascontiguousarray(np.asarray(gate, dtype=np.float32))
    bias = np.ascontiguousarray(np.asarray(bias, dtype=np.float32)).reshape(1, OUT)
    in_maps = [
        {"x": x[i * BSH:(i + 1) * BSH], "weight": weight, "gate": gate, "bias": bias}
        for i in range(NCORES)
    ]
    res = run_bass_kernel_spmd(nc, in_maps, core_ids=list(range(NCORES)), trace=trace)
    out = np.concatenate([res.results[i]["out"] for i in range(NCORES)], axis=0)
    return out, res


def kernel(x, weight, gate, bias):
    out, _ = run(x, weight, gate, bias, trace=False)
    return out
